# revision 1
# baseline (speedup 1.0000x reference)
"""Causal self-attention (B=2, T=2048, C=1024, H=16) on 8 trn2 NeuronCores.

Sharding: tensor-parallel over heads - 2 heads per core. Each core computes
its heads' qkv projection (column-split w_attn), causal attention, and a
row-split partial of the output projection; the host sums the 8 fp16
partials and adds the biases.

Design notes (driven by the TimelineSim cost model, which bills a matmul
as moving-rows x cycle regardless of K/M):
  - fp16 everywhere on the PE (1 cyc/row unconditionally; fp32r pays 4x
    below 256 moving rows; fp8 tested and rejected: 3e-2 rel err).
  - q,k produced transposed (qT/kT [128, T], moving data = x chunk,
    N=512/matmul); v produced directly in natural [token, dim] layout
    (moving data = wv, N=128) so no PE transpose of v is needed.
  - scores computed transposed per 128-key tile: sT [Tk, Tq], exp on ACT
    straight out of PSUM (scale=1/8 fused, no max pass), causally
    narrowed to the valid Tq range.
  - attn@v in natural orientation: y[tok, hd+1] accumulated over key
    tiles with lhsT = aT tile, rhs = v tile - N=65 moving rows only,
    4x fewer PE cycles than the transposed form. A ones-column in v
    accumulates the softmax denominator into psum column 64. All four
    q-tiles of a chunk accumulate in ONE psum bank per head, so group
    flags are bank-level: one start (first matmul) / stop (last).
  - diagonal masking: one constant [128,128] upper-triangular fp16 mask
    multiplied into the diagonal aT sub-block on DVE (the Pool Q7 launch
    latency + sem-blocked in-order queue stalled the diagonal attn@v).
  - normalize: reciprocal of the denominator column + per-q-tile
    broadcast multiply (DVE, psum->sbuf); fp16 PE transposes (1 cyc/row)
    then give yT [dim, tok] for the output projection, drained by ACT.
  - k-bias is softmax-invariant (adds a per-query constant) - dropped.
    v-bias commutes through softmax (weights sum to 1): folded into the
    host-side output bias as b_attn_v @ w_proj. Only the q-bias is
    applied on-device (per-partition DVE add during the psum drain).
  - software-pipelined emission: each attention chunk's tk-loop trails
    attn@v two tiles behind scores/exp, and weaves in the NEXT chunk's
    qkv units plus the PREVIOUS chunk's normalize/transpose/out-proj
    units as PE filler, so PE never waits on ACT exp or DVE drains.
    Output-projection psum drains split DVE/ACT; out partials are fp16.
"""

import sys

if "/opt/trn_rl_repo" not in sys.path:
    sys.path.insert(0, "/opt/trn_rl_repo")

import numpy as np

import concourse.bass as bass
import concourse.mybir as mybir
import concourse.tile as tile
from concourse import bacc
from concourse.bass import ds, ts
from concourse.bass_utils import run_bass_kernel_spmd

F16 = mybir.dt.float16
F32 = mybir.dt.float32
R32 = mybir.dt.float32r
U16 = mybir.dt.uint16
EXP = mybir.ActivationFunctionType.Exp
ADD = mybir.AluOpType.add
MUL = mybir.AluOpType.mult

N_CORES = 8
HD = 64   # head dim
JW = 128  # per-core qkv width: 2 heads x 64


CFG = {"weave": "jit", "ytp_pool": "psP", "fp8_out": False,
       "trail": 4, "trail_big": 5, "act_mod": 0, "fo_split": False, "atp_bufs": 6,
       "act_last": True, "fp8_v": False, "ytc_act": True}
F8 = mybir.dt.float8e4
DR = mybir.MatmulPerfMode.DoubleRow
OSCALE = 1.0 / 64.0  # wp x16, y x4 on device; undo in the psum drain


def build_program(B=2, T=2048, C=1024):
    assert T % 512 == 0 and C % 128 == 0
    NCH = T // 512   # 512-token chunks per batch
    KT = C // 128    # contraction tiles for the qkv projection
    NTK = T // 128   # 128-key tiles per batch

    nc = bacc.Bacc("TRN2", target_bir_lowering=False, debug=False)
    xt = nc.dram_tensor("xt", [C, B * T], F16, kind="ExternalInput").ap()
    if CFG["fp8_v"]:
        xt8 = nc.dram_tensor("xt8", [C, B * T], F8, kind="ExternalInput").ap()
        wv8 = nc.dram_tensor("wv8", [64, KT * 2 * JW], F8,
                             kind="ExternalInput").ap()
    else:
        xt8 = wv8 = None
    # weights pre-packed on host: row p holds [kt, 128] contiguous
    wq = nc.dram_tensor("wq", [128, KT * JW], F16, kind="ExternalInput").ap()
    wk = nc.dram_tensor("wk", [128, KT * JW], F16, kind="ExternalInput").ap()
    wv = nc.dram_tensor("wv", [128, KT * JW], F16, kind="ExternalInput").ap()
    bq = nc.dram_tensor("bq", [JW, 1], F32, kind="ExternalInput").ap()
    if CFG["fp8_out"]:
        # DoubleRow layout: dim d of the JW contraction lives at
        # (partition d % 64, k-tile d // 64); host packs wp to match.
        wp = nc.dram_tensor("wp", [64, 2 * C], F8, kind="ExternalInput").ap()
    else:
        wp = nc.dram_tensor("wp", [JW, C], F16, kind="ExternalInput").ap()
    out = nc.dram_tensor("out", [B * T, C], F16, kind="ExternalOutput").ap()

    xt_r = xt.rearrange("(kt p) t -> p kt t", p=128)
    xt8_r = xt8.rearrange("(kt t p) tok -> p kt t tok", p=64, t=2) if xt8 is not None else None

    with tile.TileContext(nc) as tc:
        _build(tc, B, T, C, NCH, KT, NTK, xt_r, wq, wk, wv, bq, wp, out,
               xt8_r, wv8)
    nc.compile()
    return nc


def _build(tc, B, T, C, NCH, KT, NTK, xt_r, wq, wk, wv, bq, wp, out,
           xt8_r=None, wv8=None):
    nc = tc.nc
    from contextlib import ExitStack

    from concourse import library_config

    nc.gpsimd.load_library(library_config.attn)

    with ExitStack() as ctx:
        const = ctx.enter_context(tc.tile_pool(name="const", bufs=1))
        wpool = ctx.enter_context(tc.tile_pool(name="wpool", bufs=1))
        pbp = ctx.enter_context(tc.tile_pool(name="pbp", bufs=1))
        xtp = ctx.enter_context(tc.tile_pool(name="xtp", bufs=2))
        atp = ctx.enter_context(tc.tile_pool(name="atp", bufs=CFG["atp_bufs"]))
        y2p = ctx.enter_context(tc.tile_pool(name="y2p", bufs=2))
        ytp = ctx.enter_context(tc.tile_pool(name="ytp", bufs=2))
        rcp = ctx.enter_context(tc.tile_pool(name="rcp", bufs=2))
        osp = ctx.enter_context(tc.tile_pool(name="osp", bufs=4))
        psS = ctx.enter_context(tc.tile_pool(name="psS", bufs=2, space="PSUM"))
        psY = ctx.enter_context(tc.tile_pool(name="psY", bufs=1, space="PSUM"))
        psP = ctx.enter_context(tc.tile_pool(name="psP", bufs=2, space="PSUM"))

        # constants: transpose identity + upper-triangular causal mask (fp16)
        ident = const.tile([128, 128], R32)
        nc.gpsimd.memset(ident[:].bitcast(mybir.dt.uint32), 0)
        nc.gpsimd.affine_select(
            out=ident[:], in_=ident[:],
            compare_op=mybir.AluOpType.not_equal, fill=1.0,
            base=0, pattern=[[-1, 128]], channel_multiplier=1,
        )
        # mask[p, c] = 1.0 if c >= p else 0  (valid: query c >= key p)
        mask = const.tile([128, 128], F16)
        nc.gpsimd.memset(mask[:].bitcast(U16), 15360)  # fp16 1.0
        nc.gpsimd.affine_select(
            out=mask[:], in_=mask[:],
            compare_op=mybir.AluOpType.is_ge, fill=0.0,
            base=0, pattern=[[1, 128]], channel_multiplier=-1,
        )
        ident16 = const.tile([128, 128], F16)
        nc.gpsimd.memset(ident16[:].bitcast(U16), 0)
        nc.gpsimd.affine_select(
            out=ident16[:], in_=ident16[:],
            compare_op=mybir.AluOpType.not_equal, fill=1.0,
            base=0, pattern=[[-1, 128]], channel_multiplier=1,
        )
        bq_sb = const.tile([JW, 1], F32)

        wq_sb = wpool.tile([128, KT, JW], F16)
        wk_sb = wpool.tile([128, KT, JW], F16)
        if CFG["fp8_v"]:
            wv_sb = wpool.tile([64, KT, 2, JW], F8)
        else:
            wv_sb = wpool.tile([128, KT, JW], F16)
        if CFG["fp8_out"]:
            wp_sb = wpool.tile([64, 2, C], F8)
        else:
            wp_sb = wpool.tile([JW, C], F16)

        # persistent per-batch tensors
        qTs, kTs, vsbs = {}, {}, {}
        for b in range(B):
            qT = pbp.tile([JW, T], F16, tag=f"qT{b}")
            kT = pbp.tile([JW, T], F16, tag=f"kT{b}")
            # v natural layout: [tok-in-tile, key tile, head, hd + ones col]
            vsb = pbp.tile([128, NTK, 2, HD + 1], F16, tag=f"vsb{b}")
            qTs[b], kTs[b], vsbs[b] = qT, kT, vsb
            nc.gpsimd.memset(vsb[:, :, :, HD : HD + 1].bitcast(U16), 15360)

        def qkv_units(b, j, pre_xt=None):
            """qkv projection for 512-token chunk j of batch b, as ~1us
            emission units so it can weave into an attention tk-loop."""
            col0 = b * T + 512 * j
            st = {}

            def u_load_q03():
                if pre_xt is not None:
                    xt_t = pre_xt
                else:
                    xt_t = xtp.tile([128, KT, 512], F16, tag="xt")
                    if j == 0:
                        for kk in range(0, KT, 2):
                            nc.sync.dma_start(xt_t[:, kk : kk + 2],
                                              xt_r[:, kk : kk + 2, ds(col0, 512)])
                    else:
                        nc.sync.dma_start(xt_t[:, 0:4],
                                          xt_r[:, 0:4, ds(col0, 512)])
                        nc.sync.dma_start(xt_t[:, 4:KT],
                                          xt_r[:, 4:KT, ds(col0, 512)])
                st["xt"] = xt_t
                if CFG["fp8_v"]:
                    xt8_t = xtp.tile([64, KT, 2, 512], F8, tag="xt8")
                    nc.sync.dma_start(xt8_t[:],
                                      xt8_r[:, :, :, ds(col0, 512)])
                    st["xt8"] = xt8_t
                psq = psP.tile([128, 512], F32, tag="p")
                st["psq"] = psq
                for kt in range(4):
                    nc.tensor.matmul(psq[:], wq_sb[:, kt], xt_t[:, kt],
                                     start=(kt == 0), stop=False)

            def u_q47():
                xt_t, psq = st["xt"], st["psq"]
                for kt in range(4, KT):
                    nc.tensor.matmul(psq[:], wq_sb[:, kt], xt_t[:, kt],
                                     start=False, stop=(kt == KT - 1))
                nc.vector.tensor_tensor(qTs[b][:, ts(j, 512)], psq[:],
                                        bq_sb[:].to_broadcast([JW, 512]), ADD)

            def u_k03():
                psk = psP.tile([128, 512], F32, tag="p")
                st["psk"] = psk
                for kt in range(4):
                    nc.tensor.matmul(psk[:], wk_sb[:, kt], st["xt"][:, kt],
                                     start=(kt == 0), stop=False)

            def u_k47():
                psk = st["psk"]
                for kt in range(4, KT):
                    nc.tensor.matmul(psk[:], wk_sb[:, kt], st["xt"][:, kt],
                                     start=False, stop=(kt == KT - 1))
                nc.vector.tensor_copy(kTs[b][:, ts(j, 512)], psk[:])

            def u_v01():
                psv = psP.tile([128, 4, 128], F32, tag="p")
                st["psv"] = psv
                for t4 in range(2):
                    for kt in range(KT):
                        if CFG["fp8_v"]:
                            nc.tensor.matmul(psv[:, t4, :],
                                             st["xt8"][0:64, kt, :, ts(t4, 128)],
                                             wv_sb[0:64, kt, :, :],
                                             start=(t4 == 0 and kt == 0),
                                             stop=False, perf_mode=DR)
                        else:
                            nc.tensor.matmul(psv[:, t4, :],
                                             st["xt"][:, kt, ts(t4, 128)],
                                             wv_sb[:, kt],
                                             start=(t4 == 0 and kt == 0),
                                             stop=False)

            def u_v23():
                psv = st["psv"]
                for t4 in range(2, 4):
                    for kt in range(KT):
                        if CFG["fp8_v"]:
                            nc.tensor.matmul(psv[:, t4, :],
                                             st["xt8"][0:64, kt, :, ts(t4, 128)],
                                             wv_sb[0:64, kt, :, :],
                                             start=False,
                                             stop=(t4 == 3 and kt == KT - 1),
                                             perf_mode=DR)
                        else:
                            nc.tensor.matmul(psv[:, t4, :],
                                             st["xt"][:, kt, ts(t4, 128)],
                                             wv_sb[:, kt],
                                             start=False,
                                             stop=(t4 == 3 and kt == KT - 1))
                if CFG["fp8_v"]:
                    nc.vector.tensor_scalar(
                        vsbs[b][:, ds(4 * j, 4), :, 0:HD],
                        psv[:].rearrange("p t4 (h d) -> p t4 h d", h=2),
                        1.0 / 16.0, None, MUL)
                else:
                    nc.vector.tensor_copy(
                        vsbs[b][:, ds(4 * j, 4), :, 0:HD],
                        psv[:].rearrange("p t4 (h d) -> p t4 h d", h=2),
                    )

            return [u_load_q03, u_q47, u_k03, u_k47, u_v01, u_v23]

        def attn_step(b, j, extra_units):
            """One pipeline step: the attention tk-loop for chunk (b, j) with
            qkv units for the next chunk and this chunk's own normalize/
            transpose/output-projection units woven in as PE filler."""
            qT, kT, vsb = qTs[b], kTs[b], vsbs[b]
            ntk = 4 * (j + 1)
            psyA = psY.tile([128, 4, HD + 1], F32, tag="yA",
                            padded_shape=[128, 4, 128])
            psyB = psY.tile([128, 4, HD + 1], F32, tag="yB",
                            padded_shape=[128, 4, 128])
            st = {}

            aTs = {}

            def s_unit(tk):
                """scores + exp + diagonal mask for key tile tk."""
                c0 = max(0, 128 * tk - 512 * j)
                pss = psS.tile([128, 2, 512], F32, tag="s")
                for h in range(2):
                    nc.tensor.matmul(
                        pss[:, h, c0:512],
                        kT[ds(HD * h, HD), ts(tk, 128)],
                        qT[ds(HD * h, HD), ds(512 * j + c0, 512 - c0)],
                        start=True, stop=True,
                    )
                aT = atp.tile([128, 2, 512], F16, tag="aT")
                aTs[tk] = aT
                nc.scalar.activation(aT[:, :, c0:512], pss[:, :, c0:512],
                                     EXP, scale=0.125)
                if tk >= 4 * j:
                    d = tk - 4 * j  # diagonal q-tile index within chunk
                    # DVE, not gpsimd: the Pool Q7 launch latency and its
                    # sem-blocked in-order queue stall the diagonal attn@v
                    for h in range(2):
                        nc.vector.tensor_tensor(
                            aT[:, h, ts(d, 128)], aT[:, h, ts(d, 128)],
                            mask[:], MUL,
                        )

            def a_unit(tk):
                """attn@v accumulation for key tile tk (runs one iteration
                behind s_unit so the exp has left the ACT queue)."""
                aT = aTs.pop(tk)
                for qq in range(4):
                    qg = 4 * j + qq  # global q-tile index
                    if qg < tk:
                        continue
                    for h, psy in ((0, psyA), (1, psyB)):
                        nc.tensor.matmul(
                            psy[:, qq, :],
                            aT[:, h, ts(qq, 128)],
                            vsb[:, tk, h, :],
                            start=(tk == 0 and qq == 0),
                            stop=(tk == ntk - 1 and qq == 3),
                        )

            def fu_norm(p0):
                """normalize q-tiles p0, p0+1 (attn@v chains stopped): DVE."""
                if "y2" not in st:
                    st["y2"] = y2p.tile([128, 4, 2, HD], F16, tag="y2", name="y2")
                y2 = st["y2"]
                for h, psy in ((0, psyA), (1, psyB)):
                    rc = rcp.tile([128, 2, 1], F32, tag=f"rc{h}")
                    nc.vector.reciprocal(rc[:], psy[:, ds(p0, 2), HD : HD + 1])
                    nc.vector.tensor_tensor(
                        y2[:, ds(p0, 2), h, :], psy[:, ds(p0, 2), 0:HD],
                        rc[:].to_broadcast([128, 2, HD]), MUL,
                    )

            def fu_transp(p0):
                """transpose q-tiles p0, p0+1 to yT layout: PE + DVE drain."""
                if CFG["fp8_out"]:
                    # split transposes land both JW halves on partitions
                    # 0..63, giving the [64, ktile, tok] DoubleRow layout
                    if "yT2" not in st:
                        st["yT2"] = ytp.tile([64, 4, 2, 128], F8, tag="yT2", name="yT28")
                    yT28 = st["yT2"]
                    yT2p8 = psP.tile([64, 2, 2, 128], R32, tag="p", name="yT2p8")
                    for iq, qq in enumerate((p0, p0 + 1)):
                        for t in range(2):
                            nc.tensor.matmul(
                                yT2p8[0:64, iq, t, :],
                                st["y2"][:, qq, t, :], ident[:],
                                is_transpose=True,
                                start=(iq == 0 and t == 0),
                                stop=(iq == 1 and t == 1))
                    nc.vector.tensor_scalar(
                        yT28[0:64, ds(p0, 2), :, :], yT2p8[0:64],
                        4.0, None, MUL)
                    return
                if "yT2" not in st:
                    yT2 = ytp.tile([128, 4, 128], F16, tag="yT2")
                    st["yT2"] = yT2
                yT2 = st["yT2"]
                yT2p = psP.tile([128, 2, 128], F16, tag="p", name="yT2p",
                                padded_shape=[128, 2, 512])
                for iq, qq in enumerate((p0, p0 + 1)):
                    nc.tensor.matmul(yT2p[:, iq, :],
                                     st["y2"][:, qq, :, :], ident16[:],
                                     is_transpose=True,
                                     start=(iq == 0), stop=(iq == 1))
                if CFG.get("ytc_act"):
                    nc.scalar.activation(yT2[:, ds(p0, 2), :], yT2p[:],
                                         mybir.ActivationFunctionType.Copy)
                else:
                    nc.vector.tensor_copy(yT2[:, ds(p0, 2), :], yT2p[:])

            def fo(qq, nhs=(0, 1)):
                row0 = b * T + 512 * j + 128 * qq
                if 0 in nhs:
                    st[f"osb{qq}"] = osp.tile([128, C], F16, tag="osb",
                                              name="osb")
                osb = st[f"osb{qq}"]
                for nh in nhs:
                    pso = psP.tile([128, 512], F32, tag="p")
                    if CFG["fp8_out"]:
                        nc.tensor.matmul(pso[:], st["yT2"][0:64, qq, :, :],
                                         wp_sb[0:64, :, ts(nh, 512)],
                                         start=True, stop=True, perf_mode=DR)
                    else:
                        nc.tensor.matmul(pso[:], st["yT2"][:, qq, :],
                                         wp_sb[:, ts(nh, 512)],
                                         start=True, stop=True)
                    if nh == 1 and ((CFG["act_mod"] and (j + qq) % CFG["act_mod"] == 0) or (CFG.get("act_last") and b == B - 1 and j == NCH - 1) or (CFG.get("act_early") and j <= 0)):
                        nc.scalar.activation(
                            osb[:, ts(nh, 512)], pso[:],
                            mybir.ActivationFunctionType.Copy,
                            scale=OSCALE if CFG["fp8_out"] else 1.0,
                        )
                    elif CFG["fp8_out"]:
                        nc.vector.tensor_scalar(
                            osb[:, ts(nh, 512)], pso[:], OSCALE, None, MUL)
                    else:
                        nc.vector.tensor_copy(osb[:, ts(nh, 512)], pso[:])
                if 1 in nhs:
                    nc.sync.dma_start(out[ds(row0, 128), :], osb[:])

            # the psum zero-region rule forbids reading psy mid-group, so
            # finish units run after the whole chunk's attn@v stops; the
            # caller weaves them into the NEXT pipeline step.
            if CFG["fo_split"]:
                finish = [
                    lambda: fu_norm(0),
                    lambda: fu_transp(0),
                    lambda: fo(0, (0,)), lambda: fo(0, (1,)),
                    lambda: fo(1, (0,)), lambda: fo(1, (1,)),
                    lambda: fu_norm(2),
                    lambda: fu_transp(2),
                    lambda: fo(2, (0,)), lambda: fo(2, (1,)),
                    lambda: fo(3, (0,)), lambda: fo(3, (1,)),
                ]
            elif CFG.get("norm_first"):
                finish = [
                    lambda: fu_norm(0),
                    lambda: fu_norm(2),
                    lambda: fu_transp(0),
                    lambda: fu_transp(2),
                    lambda: fo(0),
                    lambda: fo(1),
                    lambda: fo(2),
                    lambda: fo(3),
                ]
            else:
                finish = [
                    lambda: fu_norm(0),
                    lambda: fu_transp(0),
                    lambda: fo(0),
                    lambda: fo(1),
                    lambda: fu_norm(2),
                    lambda: fu_transp(2),
                    lambda: fo(2),
                    lambda: fo(3),
                ]
            pending = list(extra_units)
            total_fill = len(pending)
            emitted = 0
            TRAIL = CFG["trail"] if ntk > 4 else 1
            if CFG.get("trail_big") and ntk >= 12:
                TRAIL = CFG["trail_big"]
            for tk in range(ntk):
                s_unit(tk)
                target = -(-total_fill * (tk + 1) // ntk)  # ceil
                nb = CFG.get("fill_before_a", 1)
                while nb > 0 and emitted < target and pending:
                    pending.pop(0)()
                    emitted += 1
                    nb -= 1
                if tk >= TRAIL:
                    a_unit(tk - TRAIL)
                while emitted < target and pending:
                    pending.pop(0)()
                    emitted += 1
            for tk in range(ntk - TRAIL, ntk):
                a_unit(tk)
                if pending:
                    pending.pop(0)()
            for u in pending:
                u()
            return finish

        # ---- startup: stage DMAs so the first q matmuls start early ----
        # startup: interleave xt(0,0) slices with weight loads so the
        # first q matmuls can begin as soon as wq[0:2] + xt[0:2] land
        wq_r2 = wq.rearrange("p (kt j) -> p kt j", kt=KT)
        xt00 = xtp.tile([128, KT, 512], F16, tag="xt")
        nc.sync.dma_start(wq_sb[:, 0:2], wq_r2[:, 0:2])
        nc.sync.dma_start(xt00[:, 0:2], xt_r[:, 0:2, ds(0, 512)])
        nc.sync.dma_start(wq_sb[:, 2:4], wq_r2[:, 2:4])
        nc.sync.dma_start(xt00[:, 2:4], xt_r[:, 2:4, ds(0, 512)])
        nc.sync.dma_start(wq_sb[:, 4:KT], wq_r2[:, 4:KT])
        nc.sync.dma_start(xt00[:, 4:6], xt_r[:, 4:6, ds(0, 512)])
        nc.sync.dma_start(bq_sb[:], bq)
        nc.sync.dma_start(xt00[:, 6:KT], xt_r[:, 6:KT, ds(0, 512)])
        nc.sync.dma_start(wk_sb[:], wk.rearrange("p (kt j) -> p kt j", kt=KT))
        u0 = qkv_units(0, 0, pre_xt=xt00)
        u0[0]()  # first q matmuls
        u0[1]()
        if CFG["fp8_v"]:
            nc.sync.dma_start(wv_sb[:],
                              wv8.rearrange("p (kt t j) -> p kt t j",
                                            kt=KT, t=2))
        else:
            nc.sync.dma_start(wv_sb[:],
                              wv.rearrange("p (kt j) -> p kt j", kt=KT))
        u0[2]()
        u0[3]()
        if CFG["fp8_out"]:
            nc.sync.dma_start(wp_sb[:], wp.rearrange("p (t c) -> p t c", t=2))
        else:
            nc.sync.dma_start(wp_sb[:], wp)
        u0[4]()
        u0[5]()
        for u in qkv_units(1, 0):
            u()
        # just-in-time weave: qkv for step s+1's chunk fills step s, so the
        # tail steps (which have no qkv of their own left) stay PE-fed
        steps = [(b, j) for j in range(NCH) for b in range(B)]
        carried = []
        for s, (b, j) in enumerate(steps):
            if CFG["weave"] == "jit":
                if s + 1 < len(steps) and steps[s + 1][1] >= 1:
                    qk = qkv_units(*steps[s + 1])
                else:
                    qk = []
            else:
                qk = qkv_units(b, j + 1) if j + 1 < NCH else []
            # interleave the previous step's finish units with qkv units
            extra = []
            n = max(len(carried), len(qk))
            for i in range(n):
                if i < len(carried):
                    extra.append(carried[i])
                if i < len(qk):
                    extra.append(qk[i])
            carried = attn_step(b, j, extra)
        # tail: run both normalize+transpose pairs first so the four output
        # projections stream back-to-back with their DVE/ACT drains parallel
        if len(carried) == 8:
            carried = [carried[i] for i in (0, 1, 4, 5, 2, 3, 6, 7)]
        for u in carried:
            u()


def _pack_wv8(wv):
    # [C, 128] -> [64, KT, 2, 128]: contraction dim c = kt*128 + t*64 + p
    import ml_dtypes
    KT = wv.shape[0] // 128
    w = (16.0 * wv).reshape(KT, 2, 64, 128).transpose(2, 0, 1, 3)
    return np.ascontiguousarray(w.reshape(64, -1).astype(ml_dtypes.float8_e4m3))


def _pack_wp(wp):
    if CFG["fp8_out"]:
        import ml_dtypes
        return np.ascontiguousarray(
            (16.0 * wp.reshape(2, 64, -1).transpose(1, 0, 2)
             .reshape(64, -1)).astype(ml_dtypes.float8_e4m3)
        )
    return np.ascontiguousarray(wp.astype(np.float16))


def make_in_maps(x, w_attn, b_attn, w_proj):
    B, T, C = x.shape
    KT = C // 128
    x = np.asarray(x, np.float32)
    w_attn = np.asarray(w_attn, np.float32)
    b_attn = np.asarray(b_attn, np.float32)
    w_proj = np.asarray(w_proj, np.float32)
    xt = np.ascontiguousarray(
        x.transpose(2, 0, 1).reshape(C, B * T).astype(np.float16)
    )
    if CFG["fp8_v"]:
        import ml_dtypes
        xt8 = np.ascontiguousarray(
            x.transpose(2, 0, 1).reshape(C, B * T).astype(ml_dtypes.float8_e4m3)
        )

    def pack(w):  # [C, 128] -> [128, KT*128], row p = [kt, j] contiguous
        return np.ascontiguousarray(
            w.reshape(KT, 128, 128).transpose(1, 0, 2).reshape(128, KT * 128)
        ).astype(np.float16)

    in_maps = []
    for i in range(N_CORES):
        h0 = i * JW
        extra8 = (
            {"xt8": xt8,
             "wv8": _pack_wv8(w_attn[:, 2 * C + h0 : 2 * C + h0 + JW])}
            if CFG["fp8_v"] else {}
        )
        in_maps.append(
            {
                **extra8,
                "xt": xt,
                "wq": pack(w_attn[:, h0 : h0 + JW]),
                "wk": pack(w_attn[:, C + h0 : C + h0 + JW]),
                "wv": pack(w_attn[:, 2 * C + h0 : 2 * C + h0 + JW]),
                "bq": np.ascontiguousarray(
                    b_attn[h0 : h0 + JW].reshape(JW, 1).astype(np.float32)
                ),
                "wp": _pack_wp(w_proj[h0 : h0 + JW, :]),
            }
        )
    return in_maps


_PROGRAM_CACHE = {}


def _get_program(B, T, C):
    key = (B, T, C)
    if key not in _PROGRAM_CACHE:
        _PROGRAM_CACHE[key] = build_program(B, T, C)
    return _PROGRAM_CACHE[key]


def kernel(x, w_attn, b_attn, w_proj, b_proj, _trace=False):
    B, T, C = x.shape
    nc = _get_program(B, T, C)
    in_maps = make_in_maps(x, w_attn, b_attn, w_proj)
    res = run_bass_kernel_spmd(nc, in_maps, list(range(N_CORES)), trace=_trace)
    out = np.zeros((B * T, C), np.float32)
    for r in res.results:
        out += np.asarray(r["out"], np.float32)
    b_attn = np.asarray(b_attn, np.float32)
    w_proj = np.asarray(w_proj, np.float32)
    # v-bias commutes through softmax; k-bias is softmax-invariant
    out += (np.asarray(b_proj, np.float32) + b_attn[2 * C :] @ w_proj)[None, :]
    out = out.reshape(B, T, C)
    kernel.last_exec_time_ns = res.exec_time_ns
    return out

